# revision 31
# baseline (speedup 1.0000x reference)
"""Multi-head attention (B=2, S=2048, EMB=1024, H=16, hd=64) on 8 TRN2 cores.

Sharding: core c -> batch b = c//4, head-group g = c%4 (4 heads, 256 emb dims).
Per core (fp16 matmuls, fp32 psum):
  A) Q^T = Wq_g @ x_b^T   [256, 2048]   (2 passes of 4 psum banks each)
     K^T = Wk_g @ x_b^T   [256, 2048]
     V   = x_b @ Wv_g^T   [2048, 260]   (+ones column per head, s-outer)
  B) per head-pair mh (heads 2mh rows 0-63, 2mh+1 rows 64-127, concurrent
     PE row-groups), per q-segment qh (4 x 512), per key-tile t (16):
       S^T[k,q] psum; exp via custom DVE op (head A: deg-4 poly of e^(s/64)
       then x^8; 2-pass) and ACT exp (head B, paired over 2 key-tiles);
       U_aug[65, 512] += [V_h|1].T @ P^T  (row 64 = softmax sums)
     Per (mh, qh): eager normalization: r = 1/sums (custom DVE recip),
     gpsimd partition_broadcast of r, O^T = U^T * r into oT.
  C) y = O @ Wo_g^T [2048, 1024] fp16; host sums the 4 head-group partials.
"""
import numpy as np

import concourse.bass as bass
import concourse.tile as tile
from concourse import bacc, mybir
from concourse.bass_utils import run_bass_kernel_spmd

# ---------------------------------------------------------------- custom DVE
import concourse.dve_ops as _dops
from concourse.dve_ops import DveOp as _DveOp
from concourse.dve_spec import (
    C0 as _C0, C1 as _C1, C2 as _C2, C3 as _C3, One as _One, Spec as _Spec,
    Src0 as _Src0, lower as _lower, sq as _sq, _spill_c3_to_src1 as _spill,
)
from concourse.dve_uop import DveOpSpec as _DveOpSpec

# minimax deg-4 fit of e^u on u in [-0.8, 0.8], c0 constrained to 1,
# pre-scaled by 1/64: p = 1 + s(c1 + s(c2 + s(c3 + s*c4))) ~= e^(s/64);
# p^8 = exp(s/8) at ~6e-4 rms rel err (incl fp16 output rounding).
EXP_C1 = 0.015613805
EXP_C2 = 0.00012229478
EXP_C3 = 6.587349e-07
EXP_C4 = 2.425854e-09


def _ref_exp4h(in0, in1, s0, s1, imm2):
    c4 = np.asarray(in1, dtype=np.float32).reshape(in0.shape[0], -1)[:, 0:1]
    y = np.asarray(in0, dtype=np.float32)
    return 1.0 + y * (s0 + y * (s1 + y * (imm2 + y * c4)))


def _ref_sq3(in0, in1, s0, s1, imm2):
    p = np.asarray(in0, dtype=np.float32)
    return ((p * p) ** 2) ** 2


def _register(name, spec, rd1_en, perf_en=None):
    if name in _dops._SUB_OPCODE_FOR_NAME:
        return next(o for o in _dops.OPS if o.name == name)
    row = max(_dops._SUB_OPCODE_FOR_NAME.values()) + 1
    assert row < 0x20
    shas = {}
    for ver in ("v3", "v4"):
        try:
            u = _lower(spec, ver=ver)
            shas[ver] = _DveOpSpec(
                name=name, opcode=row, uops=u, rd1_en=rd1_en
            ).sha(ver)
        except Exception:
            pass
    op = _DveOp(name, spec, subdim=False, uops_sha=shas,
                perf_en=perf_en or {})
    _dops.OPS.append(op)
    _dops._SUB_OPCODE_FOR_NAME[name] = row
    return op


_y = _Src0
EXP4H = _register(
    "EXP4H_ANT",
    _Spec(body=_spill(_One + _y * (_C0 + _y * (_C1 + _y * (_C2 + _y * _C3)))),
          reference=_ref_exp4h),
    rd1_en=True,
)
SQ3 = _register(
    "SQ3_ANT", _Spec(body=_sq(_sq(_sq(_Src0))), reference=_ref_sq3),
    rd1_en=False, perf_en={"v3": True, "v4": True},
)

# ---------------------------------------------------------------- constants
F32 = mybir.dt.float32
FP16 = mybir.dt.float16
MM = FP16
EXP = mybir.ActivationFunctionType.Exp
MULT = mybir.AluOpType.mult

EMB = 1024
S = 2048
B = 2
HG = 4           # heads per core
HD = 64
CHD = HG * HD    # 256 emb dims per core
ET = EMB // 128  # 8 e-tiles
NT = S // 128    # 16 key-tiles
SH = 512         # q-segment width
NQH = S // SH    # 4

_NC = None


def _build():
    nc = bacc.Bacc("TRN2", target_bir_lowering=False, debug=False)
    xq_t = nc.dram_tensor("xq_t", [EMB, S], MM, kind="ExternalInput").ap()
    xk_t = nc.dram_tensor("xk_t", [EMB, S], MM, kind="ExternalInput").ap()
    xv_t = nc.dram_tensor("xv_t", [EMB, S], MM, kind="ExternalInput").ap()
    wq_t = nc.dram_tensor("wq_t", [EMB, CHD], MM, kind="ExternalInput").ap()
    wk_t = nc.dram_tensor("wk_t", [EMB, CHD], MM, kind="ExternalInput").ap()
    wv_t = nc.dram_tensor("wv_t", [EMB, CHD], MM, kind="ExternalInput").ap()
    wo_t = nc.dram_tensor("wo_t", [CHD, EMB], MM, kind="ExternalInput").ap()
    y = nc.dram_tensor("y", [S, EMB], FP16, kind="ExternalOutput").ap()

    with tile.TileContext(nc) as tc:
        with tc.tile_pool(name="const", bufs=1) as cpool, \
             tc.tile_pool(name="wqk", bufs=3) as wpool, \
             tc.tile_pool(name="big", bufs=1) as big, \
             tc.tile_pool(name="xp", bufs=12) as xp, \
             tc.tile_pool(name="bsb", bufs=3) as bsb, \
             tc.tile_pool(name="nsb", bufs=4) as nsb, \
             tc.tile_pool(name="yp", bufs=2) as ypool:

            warm = cpool.tile([128, 512], MM, name="warm")
            nc.vector.memset(warm[:], 0.25)
            c4t = cpool.tile([128, 1], F32, name="c4t")
            nc.vector.memset(c4t[:], EXP_C4)
            wo_sb = cpool.tile([128, 2, EMB], MM, name="wo_sb")

            qT = big.tile([128, 2, S], MM, name="qT")
            kT = big.tile([128, 2, S], MM, name="kT")
            oTs = [[big.tile([128, SH], MM, name=f"oT{m}_{g}")
                    for g in range(NQH)] for m in range(2)]
            v_sb = big.tile([128, NT, HG * (HD + 1)], MM, name="v_sb")
            nc.vector.memset(v_sb[:], 1.0)     # ones cols survive

            # ---- phase A: projections ----
            with tc.tile_pool(name="psA", bufs=1, space="PSUM") as psA:
                # HAM warmup during initial DMA wait (no data deps)
                wps = psA.tile([128, 512], F32, tag="qk", bufs=5, name="warmps")
                for i in range(10):
                    nc.tensor.matmul(wps[:], warm[:, 0:128], warm[:],
                                     start=True, stop=True)

                # projections in order Q -> V -> K: V needs ALL its x
                # tiles resident before its first matmul, so its DMA stream
                # goes second and its compute overlaps the xk DMA stream.
                def proj_qk(name, xdram, wdram, dst, x_tiles):
                    w_sb = wpool.tile([128, ET, CHD], MM, tag="w",
                                      name=f"w{name}_sb")
                    nc.sync.dma_start(
                        w_sb[:],
                        wdram.rearrange("(po pi) m -> pi po m", pi=128))
                    for e in range(len(x_tiles), ET):
                        x_t = xp.tile([128, S], MM, tag="x",
                                      name=f"x_{name}{e}")
                        nc.sync.dma_start(x_t[:], xdram[e * 128:(e + 1) * 128, :])
                        x_tiles.append(x_t)
                    for qhalf in range(2):
                        pss = [psA.tile([128, 512], F32, tag="qk", bufs=5,
                                        name=f"ps_{name}{qhalf}_{i}")
                               for i in range(4)]
                        for e in range(ET):
                            for m in range(2):
                                for qb in range(2):
                                    nc.tensor.matmul(
                                        pss[m * 2 + qb][:],
                                        w_sb[:, e, m * 128:(m + 1) * 128],
                                        x_tiles[e][:, qhalf * 1024 + qb * 512:
                                                   qhalf * 1024 + (qb + 1) * 512],
                                        start=(e == 0), stop=(e == ET - 1))
                        for m in range(2):
                            for qb in range(2):
                                cp = nc.scalar.copy if (m + qb) % 2 else \
                                    nc.vector.tensor_copy
                                o0 = qhalf * 1024 + qb * 512
                                cp(dst[:, m, o0:o0 + 512], pss[m * 2 + qb][:])

                xq0 = xp.tile([128, S], MM, tag="x", name="x_q0")
                nc.sync.dma_start(xq0[:], xq_t[0:128, :])
                proj_qk("q", xq_t, wq_t, qT, [xq0])
                proj_qk("k", xk_t, wk_t, kT, [])

                wv_sb = wpool.tile([128, ET, CHD], MM, tag="w", name="wv_sb")
                nc.sync.dma_start(
                    wv_sb[:], wv_t.rearrange("(po pi) m -> pi po m", pi=128))
                xv_tiles = []
                for e in range(ET):
                    x_t = xp.tile([128, S], MM, tag="x", name=f"x_v{e}")
                    nc.sync.dma_start(x_t[:], xv_t[e * 128:(e + 1) * 128, :])
                    xv_tiles.append(x_t)
                nc.sync.dma_start(
                    wo_sb[:], wo_t.rearrange("(ct p) n -> p ct n", p=128))
                for s in range(NT):
                    v_ps = psA.tile([128, CHD], F32, tag="psv", bufs=3,
                                    name=f"ps_v{s}")
                    for e in range(ET):
                        nc.tensor.matmul(
                            v_ps[:], xv_tiles[e][:, s * 128:(s + 1) * 128],
                            wv_sb[:, e, :],
                            start=(e == 0), stop=(e == ET - 1))
                    src = v_ps[:].rearrange("p (h d) -> p h d", d=HD)
                    dstv = v_sb[:, s, :].rearrange("p (h d) -> p h d",
                                                   d=HD + 1)[:, :, 0:HD]
                    cp = nc.scalar.copy if s % 2 else nc.vector.tensor_copy
                    cp(dstv, src)

            # ---- phase B: attention ----
            with tc.tile_pool(name="psB", bufs=1, space="PSUM") as psB:
                # exp split: ACT (native, 1147ns/tile) takes 11 key-tiles,
                # DVE (custom 2-pass, ~2440ns/tile) takes 5.
                DVE_T = {0, 3, 6, 9, 12}
                pending = None

                def emit_norm_tail(p):
                    segp, pmh, pqh = p
                    for u_c, sbr, bp in segp:
                        rb = nsb.tile([HD, SH], F32, tag=f"rb{bp}",
                                      bufs=4, name=f"rb{pmh}{pqh}{bp}")
                        nc.vector.reciprocal_approx_fast(out=rb[:], in_=sbr[:])
                        nc.gpsimd.tensor_tensor(
                            oTs[pmh][pqh][bp:bp + HD, :],
                            u_c[0:HD, :], rb[:], MULT)
                for mh in range(2):
                    hA, hB = 2 * mh, 2 * mh + 1
                    for qh in range(NQH):
                        qo = qh * SH
                        uaccA = psB.tile([HD + 1, SH], F32, tag="uaccA",
                                         bufs=1, name=f"uaccA{mh}{qh}")
                        uaccB = psB.tile([HD + 1, SH], F32, tag="uaccB",
                                         bufs=1, name=f"uaccB{mh}{qh}")
                        for t in range(NT):
                            sp2 = psB.tile([128, 2 * SH], F32, tag="sp",
                                           bufs=3, name=f"sp{mh}{qh}{t}")
                            # head A -> PE rows 0-63 -> cols 0:512 (bank b)
                            # head B -> rows 64-127 -> cols 512:1024 (bank b+1)
                            nc.tensor.matmul(
                                sp2[:, 0:SH],
                                kT[0:HD, mh, t * 128:(t + 1) * 128],
                                qT[0:HD, mh, qo:qo + SH],
                                start=True, stop=True)
                            nc.tensor.matmul(
                                sp2[:, SH:2 * SH],
                                kT[HD:128, mh, t * 128:(t + 1) * 128],
                                qT[HD:128, mh, qo:qo + SH],
                                start=True, stop=True)
                            p8 = bsb.tile([128, 2 * SH], MM, tag="p8",
                                          bufs=3, name=f"p8{mh}{qh}{t}")
                            if t in DVE_T:
                                tmp = bsb.tile([128, 2 * SH], F32, tag="tmp",
                                               bufs=2, name=f"tmp{mh}{qh}{t}")
                                nc.vector._custom_dve(
                                    EXP4H, out=tmp[:], in0=sp2[:],
                                    in1=c4t[:],
                                    s0=EXP_C1, s1=EXP_C2, imm2=EXP_C3)
                                nc.vector._custom_dve(SQ3, out=p8[:],
                                                      in0=tmp[:])
                            else:
                                nc.scalar.activation(p8[:], sp2[:], EXP,
                                                     scale=0.125)
                            nc.tensor.matmul(
                                uaccA[:],
                                v_sb[:, t, hA * (HD + 1):(hA + 1) * (HD + 1)],
                                p8[:, 0:SH],
                                start=(t == 0), stop=(t == NT - 1))
                            nc.tensor.matmul(
                                uaccB[:],
                                v_sb[:, t, hB * (HD + 1):(hB + 1) * (HD + 1)],
                                p8[:, SH:2 * SH],
                                start=(t == 0), stop=(t == NT - 1))
                        # segment end: copy U+sums out, extract sums row
                        # via DMA, recip the [1,512] row (DVE; deferred one
                        # segment so it never blocks the exp pipeline). The
                        # gpsimd broadcast+multiply are deferred with it.
                        seg = []
                        for u_c_name, uacc, bp in (("uA", uaccA, 0),
                                                   ("uB", uaccB, HD)):
                            u_c = nsb.tile([HD + 1, SH], F32, tag=u_c_name,
                                           bufs=6, name=f"{u_c_name}{mh}{qh}")
                            nc.scalar.copy(u_c[:], uacc[:])
                            rs0 = nsb.tile([1, SH], F32, tag=f"rs{bp}",
                                           bufs=6, name=f"rs{mh}{qh}{bp}")
                            nc.sync.dma_start(rs0[:], u_c[HD:HD + 1, :])
                            sbr = nsb.tile([HD, SH], F32, tag=f"sbr{bp}",
                                           bufs=6, name=f"sbr{mh}{qh}{bp}")
                            nc.gpsimd.partition_broadcast(sbr[:], rs0[:])
                            seg.append((u_c, sbr, bp))
                        if pending is not None:
                            emit_norm_tail(pending)
                        pending = (seg, mh, qh)

                emit_norm_tail(pending)

            # ---- phase C: output projection ----
            with tc.tile_pool(name="psY", bufs=1, space="PSUM") as psY:
                for s in range(NT):
                    y_ps = psY.tile([128, EMB], F32, tag="yps", bufs=3,
                                    name=f"yps{s}")
                    for nb in range(2):
                        for ct in range(2):
                            nc.tensor.matmul(
                                y_ps[:, nb * 512:(nb + 1) * 512],
                                oTs[ct][s // 4][:, (s % 4) * 128:
                                                (s % 4 + 1) * 128],
                                wo_sb[:, ct, nb * 512:(nb + 1) * 512],
                                start=(ct == 0), stop=(ct == 1))
                    y_sb = ypool.tile([128, EMB], MM, tag="ysb", bufs=4,
                                      name=f"ysb{s}")
                    cp = nc.scalar.copy if s % 2 else nc.vector.tensor_copy
                    cp(y_sb[:], y_ps[:])
                    nc.sync.dma_start(y[s * 128:(s + 1) * 128, :], y_sb[:])

    nc.compile()
    return nc


def get_nc():
    global _NC
    if _NC is None:
        _NC = _build()
    return _NC


def make_in_maps(query, key, value, Wq, Wk, Wv, Wo):
    np_dt = np.float16
    query = np.asarray(query, dtype=np.float32)
    key = np.asarray(key, dtype=np.float32)
    value = np.asarray(value, dtype=np.float32)
    Wq = np.asarray(Wq, dtype=np.float32)
    Wk = np.asarray(Wk, dtype=np.float32)
    Wv = np.asarray(Wv, dtype=np.float32)
    Wo = np.asarray(Wo, dtype=np.float32)
    xt = {(n, b): np.ascontiguousarray(x[b].T).astype(np_dt)
          for n, x in (("q", query), ("k", key), ("v", value))
          for b in range(B)}
    in_maps = []
    for c in range(8):
        b, g = divmod(c, 4)
        hs = slice(g * CHD, (g + 1) * CHD)
        in_maps.append({
            "xq_t": xt[("q", b)],
            "xk_t": xt[("k", b)],
            "xv_t": xt[("v", b)],
            "wq_t": np.ascontiguousarray(Wq[hs, :].T).astype(np_dt),
            "wk_t": np.ascontiguousarray(Wk[hs, :].T).astype(np_dt),
            "wv_t": np.ascontiguousarray(Wv[hs, :].T).astype(np_dt),
            "wo_t": np.ascontiguousarray(Wo[:, hs].T).astype(np_dt),
        })
    return in_maps


def gather(results):
    out = np.zeros((B, S, EMB), dtype=np.float32)
    for c in range(8):
        out[c // 4] += results[c]["y"].astype(np.float32)
    return out


def kernel(**inputs) -> np.ndarray:
    nc = get_nc()
    in_maps = make_in_maps(**inputs)
    res = run_bass_kernel_spmd(nc, in_maps, core_ids=list(range(8)))
    return gather(res.results)
